# revision 3
# baseline (speedup 1.0000x reference)
"""Embedding gather-sum kernel V4 for Trainium2 (8 NeuronCores, SPMD).

Problem (nn_UserLinearUpscaler):
    out[b, s, :] = sum_k W[:, ids[b, s, k]] + bias
    B=1024, S=50, K=20, E=64, V=100000, f32 weights, integer ids.

Data-parallel over batch: each core handles 128 batch rows (6400 tokens,
128000 row lookups).  W.T is replicated per core in DRAM as a bf16 table
with rows padded to 128 elements so each gather element is 256 B.

Differences vs the old f32 kernel (1.14 ms):
  - bf16 one-hot matmuls (fp32 ran at 1/4 PE rate and was the real
    bottleneck per the timeline model: PE 97.6% busy).
  - chunk = 256 tokens so token ids are exact in bf16 for the is_equal
    S-build (bf16 represents integers <= 256 exactly).
  - gathers spread across 4 SWDGE queues (measured ~2.6 ns/idx vs 4.2).
  - num_idxs_reg is read from a per-core counts tensor at runtime via
    value_load, so the NEFF does not depend on the input ids (no
    input-triggered recompiles) and each core pays only its true
    descriptor count.
"""

import numpy as np
import ml_dtypes

import concourse.bass as bass
import concourse.tile as tile
from concourse import bacc, mybir
from concourse.bass_utils import run_bass_kernel_spmd

B, S, K, E, V = 1024, 50, 20, 64, 100000
N_CORES = 8
P = 128
TOK_CORE = B // N_CORES * S          # 6400 tokens per core

T4 = 256                             # tokens per chunk (bf16-exact ids)
CH4 = TOK_CORE // T4                 # 25 chunks
SLOTS4 = T4 * K                      # 5120 lookup slots per chunk

RANGE_BASES = [0, 32768, 65536, 98304]
RANGE_SIZES = [32768, 32768, 32768, V - 98304]
# static padded per-range list lengths (multiples of 128); binomial mean for
# ranges 0-2 is 5120*32768/100000 = 1678 (sigma ~34); 1920 = 7.1 sigma.
N_SLOTS4 = (1920, 1920, 1920, 256)
NBLK4 = tuple(n // P for n in N_SLOTS4)
NBLK_TOT4 = sum(NBLK4)               # 47 blocks per chunk
NW16_4 = sum(N_SLOTS4) // 16

DMA_SCRATCH = 32768

_cache: dict = {}


def _build_v4(n_repeat=1):
    nc = bacc.Bacc("TRN2", target_bir_lowering=False, debug=False,
                   num_devices=N_CORES,
                   dynamic_dma_scratch_size=DMA_SCRATCH,
                   num_swdge_queues=4)
    wtp = nc.dram_tensor("wtp", [V, P], mybir.dt.bfloat16,
                         kind="ExternalInput")
    gidx = nc.dram_tensor("gidx", [CH4, P, NW16_4], mybir.dt.int16,
                          kind="ExternalInput")
    tokf = nc.dram_tensor("tokf", [CH4, P, NBLK_TOT4], mybir.dt.bfloat16,
                          kind="ExternalInput")
    iota = nc.dram_tensor("iota", [P, T4], mybir.dt.bfloat16,
                          kind="ExternalInput")
    biasc = nc.dram_tensor("biasc", [E, 1], mybir.dt.float32,
                           kind="ExternalInput")
    y = nc.dram_tensor("y", [CH4, E, T4], mybir.dt.float32,
                       kind="ExternalOutput")

    with tile.TileContext(nc) as tc:
        with (
            tc.tile_pool(name="idxp", bufs=2) as idxp,
            tc.tile_pool(name="constp", bufs=1) as constp,
            tc.tile_pool(name="cgp", bufs=2) as cgp,
            tc.tile_pool(name="sp", bufs=6) as sp,
            tc.tile_pool(name="psump", bufs=2, space="PSUM") as psump,
            tc.tile_pool(name="evp", bufs=2) as evp,
        ):
            iota_t = constp.tile([P, T4], mybir.dt.bfloat16)
            nc.sync.dma_start(out=iota_t[:, :], in_=iota[:, :])
            biasc_t = constp.tile([E, 1], mybir.dt.float32)
            nc.sync.dma_start(out=biasc_t[:, :], in_=biasc[:, :])

            for _ in range(n_repeat):
                for c in range(CH4):
                    gidx_t = idxp.tile([P, NW16_4], mybir.dt.int16,
                                       tag="gidx")
                    nc.sync.dma_start(out=gidx_t[:, :], in_=gidx[c])
                    tokf_t = idxp.tile([P, NBLK_TOT4], mybir.dt.bfloat16,
                                       tag="tokf")
                    nc.sync.dma_start(out=tokf_t[:, :], in_=tokf[c])

                    cgs = []
                    off = 0
                    for r in range(4):
                        n_r = N_SLOTS4[r]
                        cg = cgp.tile([P, n_r // P, P], mybir.dt.bfloat16,
                                      tag=f"cg{r}")
                        nc.gpsimd.dma_gather(
                            out_ap=cg[:, :, :],
                            in_ap=wtp[RANGE_BASES[r]:
                                      RANGE_BASES[r] + RANGE_SIZES[r], :],
                            idxs_ap=gidx_t[:, off:off + n_r // 16],
                            num_idxs=n_r,
                            num_idxs_reg=n_r,
                            elem_size=P,
                            single_packet=False,
                            queue_num=(c * 4 + r) % 4,
                        )
                        cgs.append(cg)
                        off += n_r // 16

                    psum = psump.tile([E, T4], mybir.dt.float32, tag="ps")
                    blk = 0
                    for r in range(4):
                        for bb in range(NBLK4[r]):
                            s_t = sp.tile([P, T4], mybir.dt.bfloat16, tag="S")
                            nc.vector.tensor_tensor(
                                out=s_t[:, :],
                                in0=tokf_t[:, blk:blk + 1].to_broadcast([P, T4]),
                                in1=iota_t[:, :],
                                op=mybir.AluOpType.is_equal)
                            nc.tensor.matmul(
                                out=psum[:, :],
                                lhsT=cgs[r][:, bb, 0:E],
                                rhs=s_t[:, :],
                                start=(blk == 0),
                                stop=(blk == NBLK_TOT4 - 1))
                            blk += 1

                    ev = evp.tile([E, T4], mybir.dt.float32, tag="ev")
                    nc.vector.tensor_tensor(
                        out=ev[:, :], in0=psum[:, :],
                        in1=biasc_t[:, 0:1].to_broadcast([E, T4]),
                        op=mybir.AluOpType.add)
                    nc.sync.dma_start(out=y[c], in_=ev[:, :])
    nc.compile()
    return nc


def _wrap16(flat: np.ndarray) -> np.ndarray:
    n = flat.shape[0]
    blk = flat.reshape(n // 16, 16).T            # [16, n/16]
    return np.tile(blk, (8, 1))


def _build_indices_v4(ids_core: np.ndarray):
    """ids_core: [TOK_CORE, K] int32 ->
    (gidx [CH4, P, NW16_4] int16, tokf [CH4, P, NBLK_TOT4] bf16)."""
    gidx = np.zeros((CH4, P, NW16_4), np.int16)
    tokf = np.zeros((CH4, P, NBLK_TOT4), ml_dtypes.bfloat16)
    tok_of_slot = np.arange(SLOTS4) // K
    for c in range(CH4):
        flat = ids_core[c * T4:(c + 1) * T4].reshape(-1)      # [SLOTS4]
        rng_id = flat >> 15
        local = flat & 32767
        off = 0
        boff = 0
        for r in range(4):
            sel = np.nonzero(rng_id == r)[0]
            n_r = N_SLOTS4[r]
            n = sel.shape[0]
            if n > n_r:
                raise OverflowError(f"range {r}: {n} > {n_r}")
            g = np.zeros(n_r, np.int16)         # filler gathers row 0
            g[:n] = local[sel]
            tf = np.full(n_r, -1.0, ml_dtypes.bfloat16)  # pads select nothing
            tf[:n] = tok_of_slot[sel]
            gidx[c, :, off:off + n_r // 16] = _wrap16(g)
            tokf[c, :, boff:boff + n_r // P] = tf.reshape(n_r // P, P).T
            off += n_r // 16
            boff += n_r // P
    return gidx, tokf


def _make_wtp(W: np.ndarray) -> np.ndarray:
    wtp = np.zeros((V, P), ml_dtypes.bfloat16)
    wtp[:, :E] = W.T.astype(ml_dtypes.bfloat16)
    return wtp


def _make_iota() -> np.ndarray:
    return np.ascontiguousarray(np.broadcast_to(
        np.arange(T4, dtype=np.float32), (P, T4))).astype(ml_dtypes.bfloat16)


def kernel(content_input: np.ndarray, W: np.ndarray, b: np.ndarray) -> np.ndarray:
    ids = np.ascontiguousarray(content_input).astype(np.int32).reshape(B * S, K)
    wtp = _make_wtp(W)
    iota = _make_iota()
    biasc = np.ascontiguousarray(b.astype(np.float32).reshape(E, 1))

    if "nc4" not in _cache:
        _cache["nc4"] = _build_v4()
    nc = _cache["nc4"]

    in_maps = []
    for i in range(N_CORES):
        ids_core = ids[i * TOK_CORE:(i + 1) * TOK_CORE]
        gidx, tokf = _build_indices_v4(ids_core)
        in_maps.append({"wtp": wtp, "gidx": gidx, "tokf": tokf,
                        "iota": iota, "biasc": biasc})
    res = run_bass_kernel_spmd(nc, in_maps, core_ids=list(range(N_CORES)))
    out = np.concatenate(
        [res.results[i]["y"].transpose(0, 2, 1).reshape(TOK_CORE, E)
         for i in range(N_CORES)],
        axis=0)
    return out.reshape(B, S, E)


# revision 4
# speedup vs baseline: 1.3369x; 1.3369x over previous
"""Embedding gather-sum kernel V4 for Trainium2 (8 NeuronCores, SPMD).

Problem (nn_UserLinearUpscaler):
    out[b, s, :] = sum_k W[:, ids[b, s, k]] + bias
    B=1024, S=50, K=20, E=64, V=100000, f32 weights, integer ids.

Data-parallel over batch: each core handles 128 batch rows (6400 tokens,
128000 row lookups).  W.T is replicated per core in DRAM as a bf16 table
with rows padded to 128 elements so each gather element is 256 B.

Differences vs the old f32 kernel (1.14 ms -> ~0.41 ms):
  - bf16 one-hot matmuls (fp32 ran at 1/4 PE rate and was the real
    bottleneck per the timeline model: PE 97.6% busy).
  - chunk = 256 tokens so token ids are exact in bf16 for the is_equal
    S-build (bf16 represents integers <= 256 exactly).
  - gathers spread across 4 SWDGE queues.
  - fully static NEFF: num_idxs_reg == num_idxs with zero-filled index
    padding, so no input-dependent recompiles.

Rejected variants (all measured SLOWER on real HW despite better
cost-model predictions -- keep these out):
  - tensor_scalar S-build (cost model says DVE 4x mode): 1.10 ms.
  - 4-block-batched S-build via stride-0 mid-dim broadcast: 543 us.
  - T=128 or T=160 chunks (less DVE work each): 645-743 us.
  - gathers packed 5 chunks per instruction: no gain, part of the
    645 us regression.
"""

import numpy as np
import ml_dtypes

import concourse.bass as bass
import concourse.tile as tile
from concourse import bacc, mybir
from concourse.bass_utils import run_bass_kernel_spmd

B, S, K, E, V = 1024, 50, 20, 64, 100000
N_CORES = 8
P = 128
TOK_CORE = B // N_CORES * S          # 6400 tokens per core

T4 = 256                             # tokens per chunk (bf16-exact ids)
CH4 = TOK_CORE // T4                 # 25 chunks
SLOTS4 = T4 * K                      # 5120 lookup slots per chunk

RANGE_BASES = [0, 32768, 65536, 98304]
RANGE_SIZES = [32768, 32768, 32768, V - 98304]
# static padded per-range list lengths (multiples of 128); binomial mean for
# ranges 0-2 is 5120*32768/100000 = 1678 (sigma ~34); 1920 = 7.1 sigma.
N_SLOTS4 = (1920, 1920, 1920, 256)
NBLK4 = tuple(n // P for n in N_SLOTS4)
NBLK_TOT4 = sum(NBLK4)               # 47 blocks per chunk
NW16_4 = sum(N_SLOTS4) // 16

DMA_SCRATCH = 32768

_cache: dict = {}


def _build_v4(n_repeat=1):
    nc = bacc.Bacc("TRN2", target_bir_lowering=False, debug=False,
                   num_devices=N_CORES,
                   dynamic_dma_scratch_size=DMA_SCRATCH,
                   num_swdge_queues=4)
    wtp = nc.dram_tensor("wtp", [V, P], mybir.dt.bfloat16,
                         kind="ExternalInput")
    gidx = nc.dram_tensor("gidx", [CH4, P, NW16_4], mybir.dt.int16,
                          kind="ExternalInput")
    tokf = nc.dram_tensor("tokf", [CH4, P, NBLK_TOT4], mybir.dt.bfloat16,
                          kind="ExternalInput")
    iota = nc.dram_tensor("iota", [P, T4], mybir.dt.bfloat16,
                          kind="ExternalInput")
    biasc = nc.dram_tensor("biasc", [E, 1], mybir.dt.float32,
                           kind="ExternalInput")
    y = nc.dram_tensor("y", [CH4, E, T4], mybir.dt.float32,
                       kind="ExternalOutput")

    with tile.TileContext(nc) as tc:
        with (
            tc.tile_pool(name="idxp", bufs=2) as idxp,
            tc.tile_pool(name="constp", bufs=1) as constp,
            tc.tile_pool(name="cgp", bufs=2) as cgp,
            tc.tile_pool(name="sp", bufs=6) as sp,
            tc.tile_pool(name="psump", bufs=2, space="PSUM") as psump,
            tc.tile_pool(name="evp", bufs=2) as evp,
        ):
            iota_t = constp.tile([P, T4], mybir.dt.bfloat16)
            nc.sync.dma_start(out=iota_t[:, :], in_=iota[:, :])
            biasc_t = constp.tile([E, 1], mybir.dt.float32)
            nc.sync.dma_start(out=biasc_t[:, :], in_=biasc[:, :])

            for _ in range(n_repeat):
                for c in range(CH4):
                    gidx_t = idxp.tile([P, NW16_4], mybir.dt.int16,
                                       tag="gidx")
                    nc.sync.dma_start(out=gidx_t[:, :], in_=gidx[c])
                    tokf_t = idxp.tile([P, NBLK_TOT4], mybir.dt.bfloat16,
                                       tag="tokf")
                    nc.sync.dma_start(out=tokf_t[:, :], in_=tokf[c])

                    cgs = []
                    off = 0
                    for r in range(4):
                        n_r = N_SLOTS4[r]
                        cg = cgp.tile([P, n_r // P, P], mybir.dt.bfloat16,
                                      tag=f"cg{r}")
                        nc.gpsimd.dma_gather(
                            out_ap=cg[:, :, :],
                            in_ap=wtp[RANGE_BASES[r]:
                                      RANGE_BASES[r] + RANGE_SIZES[r], :],
                            idxs_ap=gidx_t[:, off:off + n_r // 16],
                            num_idxs=n_r,
                            num_idxs_reg=n_r,
                            elem_size=P,
                            single_packet=False,
                            queue_num=(c * 4 + r) % 4,
                        )
                        cgs.append(cg)
                        off += n_r // 16

                    psum = psump.tile([E, T4], mybir.dt.float32, tag="ps")
                    blk = 0
                    for r in range(4):
                        for bb in range(NBLK4[r]):
                            s_t = sp.tile([P, T4], mybir.dt.bfloat16, tag="S")
                            nc.vector.tensor_tensor(
                                out=s_t[:, :],
                                in0=tokf_t[:, blk:blk + 1].to_broadcast([P, T4]),
                                in1=iota_t[:, :],
                                op=mybir.AluOpType.is_equal)
                            nc.tensor.matmul(
                                out=psum[:, :],
                                lhsT=cgs[r][:, bb, 0:E],
                                rhs=s_t[:, :],
                                start=(blk == 0),
                                stop=(blk == NBLK_TOT4 - 1))
                            blk += 1

                    ev = evp.tile([E, T4], mybir.dt.float32, tag="ev")
                    nc.vector.tensor_tensor(
                        out=ev[:, :], in0=psum[:, :],
                        in1=biasc_t[:, 0:1].to_broadcast([E, T4]),
                        op=mybir.AluOpType.add)
                    nc.sync.dma_start(out=y[c], in_=ev[:, :])
    nc.compile()
    return nc


def _wrap16(flat: np.ndarray) -> np.ndarray:
    n = flat.shape[0]
    blk = flat.reshape(n // 16, 16).T            # [16, n/16]
    return np.tile(blk, (8, 1))


def _build_indices_v4(ids_core: np.ndarray):
    """ids_core: [TOK_CORE, K] int32 ->
    (gidx [CH4, P, NW16_4] int16, tokf [CH4, P, NBLK_TOT4] bf16)."""
    gidx = np.zeros((CH4, P, NW16_4), np.int16)
    tokf = np.zeros((CH4, P, NBLK_TOT4), ml_dtypes.bfloat16)
    tok_of_slot = np.arange(SLOTS4) // K
    for c in range(CH4):
        flat = ids_core[c * T4:(c + 1) * T4].reshape(-1)      # [SLOTS4]
        rng_id = flat >> 15
        local = flat & 32767
        off = 0
        boff = 0
        for r in range(4):
            sel = np.nonzero(rng_id == r)[0]
            n_r = N_SLOTS4[r]
            n = sel.shape[0]
            if n > n_r:
                raise OverflowError(f"range {r}: {n} > {n_r}")
            g = np.zeros(n_r, np.int16)         # filler gathers row 0
            g[:n] = local[sel]
            tf = np.full(n_r, -1.0, ml_dtypes.bfloat16)  # pads select nothing
            tf[:n] = tok_of_slot[sel]
            gidx[c, :, off:off + n_r // 16] = _wrap16(g)
            tokf[c, :, boff:boff + n_r // P] = tf.reshape(n_r // P, P).T
            off += n_r // 16
            boff += n_r // P
    return gidx, tokf


def _make_wtp(W: np.ndarray) -> np.ndarray:
    wtp = np.zeros((V, P), ml_dtypes.bfloat16)
    wtp[:, :E] = W.T.astype(ml_dtypes.bfloat16)
    return wtp


def _make_iota() -> np.ndarray:
    return np.ascontiguousarray(np.broadcast_to(
        np.arange(T4, dtype=np.float32), (P, T4))).astype(ml_dtypes.bfloat16)


def kernel(content_input: np.ndarray, W: np.ndarray, b: np.ndarray) -> np.ndarray:
    ids = np.ascontiguousarray(content_input).astype(np.int32).reshape(B * S, K)
    wtp = _make_wtp(W)
    iota = _make_iota()
    biasc = np.ascontiguousarray(b.astype(np.float32).reshape(E, 1))

    if "nc4" not in _cache:
        _cache["nc4"] = _build_v4()
    nc = _cache["nc4"]

    in_maps = []
    for i in range(N_CORES):
        ids_core = ids[i * TOK_CORE:(i + 1) * TOK_CORE]
        gidx, tokf = _build_indices_v4(ids_core)
        in_maps.append({"wtp": wtp, "gidx": gidx, "tokf": tokf,
                        "iota": iota, "biasc": biasc})
    res = run_bass_kernel_spmd(nc, in_maps, core_ids=list(range(N_CORES)))
    out = np.concatenate(
        [res.results[i]["y"].transpose(0, 2, 1).reshape(TOK_CORE, E)
         for i in range(N_CORES)],
        axis=0)
    return out.reshape(B, S, E)
